# revision 1
# baseline (speedup 1.0000x reference)
"""DTS-SNN 2D Trainium2 kernel (8 NeuronCores, batch-data-parallel).

Reference math:
  e in {0,1}; tr1/tr2 leaky traces; enc = blockconv4x4(unfold3x3((tr1-tr2)*SCALE))
  m_in LIF -> s_in; c_h = c_h*ds + s_in@Wh.T; m_h LIF -> s_h;
  c_o = c_o*ds + s_h@Wo.T; m_o LIF -> s_o; out = sum_t(s_o)/T

Hardware restructuring (exactness ~1e-7, required: SNN flips spikes at ~1e-5):
  EC[t] = conv(e[t])  on PE as 16 matmuls/group: fp16 hi + fp16 lo stationaries
          (both scaled 2^6; evacuation copy applies 2^-6), moving = binary
          events (exact fp16), contraction over H, band-clipped stationaries.
  a1/a2 = per-feature leaky scans of EC via tensor_tensor_scan, segmented by a
          decay pattern with 0 at segment starts; a2 computed negated
          (op1=subtract) so enc[t] = a1[t] + a2n[t].
  m_in scan (t-sequential, DVE): x=(m<TH)*m; m'=x*dm+a1[t]; m'+=a2n[t];
          spike via ACT Sign(m'-TH) in {-1,+1} stored fp16.
  proj = Ssgn @ (Wh.T/2) + colsum(Wh)/2  -- one big matmul over all (b,t),
          fp16 hi/lo split (lo scaled 2^11, separate PSUM bank, combined once).
  c_h/c_o via tensor_tensor_scan after a PE transpose (csum bias folded into
          the PSUM-evacuation activation); m_h/m_o scans like m_in.

Sharding: batch 16 -> 2 per core; all weights replicated (hint-compliant).
"""

import numpy as np

import concourse.bacc as bacc
import concourse.mybir as mybir
import concourse.tile as tile
from concourse.bass_utils import run_bass_kernel_spmd

# ---- model constants -------------------------------------------------------
B, T, H, W = 16, 60, 128, 128
NCORES = 8
BL = B // NCORES
HID, OUT = 512, 11
NCH = 96                    # feature chunks, one per (dj, j); 96 feats each
TH = 0.3
SCALE = 0.5
d1 = float(np.exp(-1.0 / 20.0))
d2 = float(np.exp(-1.0 / 5.0))
dm = float(np.exp(-1.0 / 20.0))
ds = float(np.exp(-1.0 / 5.0))
CONV_SC = 2.0 ** 6          # conv stationaries pre-scaled; evac applies 2^-6
LO_SC = 2.0 ** 11           # lo-residual scale for the hidden/output weights
FW = 136                    # frame: 4 residue planes x 34 (margins baked on host)
NFR = BL * T * 2            # 240 image planes per core
FGRP = 15                   # tau frames per conv psum group
NFG = T // FGRP
FB = 6                      # feature blocks = (dj, jhalf)
CHB = NCH // FB             # 16 chunks per feature block
BT = BL * T
f16 = mybir.dt.float16
f32 = mybir.dt.float32
A_ = mybir.AluOpType
F_ = mybir.ActivationFunctionType

_CACHE: dict = {}


def _build_program(debug_taps=False):
    nc = bacc.Bacc("TRN2", target_bir_lowering=False, debug=True)

    ev_d = nc.dram_tensor("ev", [128, BL, T, 2, 4, 34], f16, kind="ExternalInput")
    ahi_d = nc.dram_tensor("ahi", [8, 128, 96], f16, kind="ExternalInput")
    alo_d = nc.dram_tensor("alo", [8, 128, 96], f16, kind="ExternalInput")
    whl_d = nc.dram_tensor("whl", [NCH // 2, 96, 4 * HID], f16, kind="ExternalInput")
    csh_d = nc.dram_tensor("csh", [4, 128], f32, kind="ExternalInput")
    wohi_d = nc.dram_tensor("wohi", [4, 128, OUT], f16, kind="ExternalInput")
    wolo_d = nc.dram_tensor("wolo", [4, 128, OUT], f16, kind="ExternalInput")
    cso_d = nc.dram_tensor("cso", [OUT], f32, kind="ExternalInput")
    id_d = nc.dram_tensor("ident", [128, 128], f32, kind="ExternalInput")
    out_d = nc.dram_tensor("out", [BL, OUT], f32, kind="ExternalOutput")
    taps = {}
    if debug_taps:
        for nm, shp in [("ec", [96, NCH * BT]), ("a1", [96, NCH * BT]),
                        ("a2n", [96, NCH * BT]),
                        ("pj", [BT, HID]), ("ch", [128, 4 * BT]),
                        ("sh", [128, 4 * BT]), ("po", [OUT, BT])]:
            taps[nm] = nc.dram_tensor("tap_" + nm, shp, f32, kind="ExternalOutput")
        taps["sg"] = nc.dram_tensor("tap_sg", [96, NCH * BT], f16, kind="ExternalOutput")

    with tile.TileContext(nc) as tc:
        with (
            tc.tile_pool(name="ev", bufs=1) as evp,
            tc.tile_pool(name="const", bufs=1) as cst,
            tc.tile_pool(name="acc", bufs=1) as accp,
            tc.tile_pool(name="pat", bufs=1) as patp,
            tc.tile_pool(name="state", bufs=1) as stp,
            tc.tile_pool(name="w", bufs=4) as wp,
            tc.tile_pool(name="cpsum", bufs=2, space="PSUM") as cps,
            tc.tile_pool(name="mpsum", bufs=1, space="PSUM") as mps,
            tc.tile_pool(name="tpsum", bufs=2, space="PSUM") as tps,
        ):
            # ---------------- constants / weights in SBUF ----------------
            ahi = cst.tile([128, 8 * 96], f16)
            alo = cst.tile([128, 8 * 96], f16)
            nc.sync.dma_start(ahi[:].rearrange("p (k m) -> p k m", k=8),
                              ahi_d[:].rearrange("k p m -> p k m"))
            nc.sync.dma_start(alo[:].rearrange("p (k m) -> p k m", k=8),
                              alo_d[:].rearrange("k p m -> p k m"))
            csh = cst.tile([128, 4], f32)
            nc.sync.dma_start(csh[:], csh_d[:].rearrange("k p -> p k"))
            wohi = cst.tile([128, 4 * OUT], f16)
            wolo = cst.tile([128, 4 * OUT], f16)
            nc.sync.dma_start(wohi[:].rearrange("p (k m) -> p k m", k=4),
                              wohi_d[:].rearrange("k p m -> p k m"))
            nc.sync.dma_start(wolo[:].rearrange("p (k m) -> p k m", k=4),
                              wolo_d[:].rearrange("k p m -> p k m"))
            cso = cst.tile([OUT, 1], f32)
            nc.sync.dma_start(cso[:], cso_d[:].rearrange("(p o) -> p o", o=1))
            ident = cst.tile([128, 128], f32)
            nc.sync.dma_start(ident[:], id_d[:])
            negTH = cst.tile([128, 1], f32)
            nc.vector.memset(negTH[:], -TH)

            # ---------------- events -> SBUF (h on partitions) -----------
            EV = evp.tile([128, NFR * FW], f16)
            ev3 = EV[:].rearrange("p (f w) -> p f w", f=NFR)
            ev6 = EV[:].rearrange("p (b t c r j) -> p b t c r j", b=BL, t=T, c=2, r=4)
            src = ev_d[:].rearrange("p b t c r j -> p (b t c) (r j)")
            NDMA = 8
            FPD = NFR // NDMA
            for k in range(NDMA):
                nc.sync.dma_start(
                    ev3[:, k * FPD : (k + 1) * FPD, :],
                    src[:, k * FPD : (k + 1) * FPD, :],
                )

            # ---------------- filter patterns (built once) ---------------
            p1 = patp.tile([96, CHB * BL * T], f32)
            p2 = patp.tile([96, CHB * BL * T], f32)
            seed1 = cst.tile([96, T], f32)
            seed2 = cst.tile([96, T], f32)
            nc.vector.memset(seed1[:, 0:1], 0.0)
            nc.vector.memset(seed1[:, 1:T], d1)
            nc.vector.memset(seed2[:, 0:1], 0.0)
            nc.vector.memset(seed2[:, 1:T], d2)
            for s in range(CHB * BL):
                nc.scalar.copy(p1[:, s * T : (s + 1) * T], seed1[:])
                nc.scalar.copy(p2[:, s * T : (s + 1) * T], seed2[:])

            # ---------------- conv + temporal filters --------------------
            # a1 free layout: (chunk=(dj,jh,j) 96, b 2, t 60)
            a1 = accp.tile([96, NCH * BT], f32)
            a1v = a1[:].rearrange("p (k b t) -> p k b t", k=NCH, b=BL)
            BLKF = CHB * BL * T
            for dj in range(3):
                for b in range(BL):
                    for g in range(NFG):
                        ps = cps.tile([96, FGRP * 32], f32)
                        t0 = g * FGRP
                        first = True
                        for c in range(2):
                            for q in range(4):
                                r0 = (q + dj - 1) % 4
                                j0 = (q + dj - 1) // 4
                                rhs = ev6[:, b, t0 : t0 + FGRP, c, r0,
                                          j0 + 1 : j0 + 33]
                                for wt in (ahi, alo):
                                    nc.tensor.matmul(
                                        ps[:],
                                        wt[:, (c * 4 + q) * 96 : (c * 4 + q + 1) * 96],
                                        rhs,
                                        start=first,
                                        stop=(c == 1 and q == 3 and wt is alo),
                                    )
                                    first = False
                        dst = a1v[:, dj * 32 : dj * 32 + 32, b, t0 : t0 + FGRP]
                        srcp = ps[:].rearrange("p (t j) -> p j t", t=FGRP)
                        nc.scalar.activation(dst, srcp, F_.Copy, scale=1.0 / CONV_SC)
            for fb in range(FB):
                blk = slice(fb * BLKF, (fb + 1) * BLKF)
                if debug_taps:
                    nc.sync.dma_start(taps["ec"][:, blk], a1[:, blk])
                a2nb = accp.tile([96, BLKF], f32, tag="a2nb", bufs=2, name="a2nb")
                nc.vector.tensor_tensor_scan(          # a2nb = -a2
                    a2nb[:], p2[:], a1[:, blk], 0.0, op0=A_.mult, op1=A_.subtract)
                nc.vector.tensor_tensor_scan(          # a1 (in place over EC)
                    a1[:, blk], p1[:], a1[:, blk], 0.0, op0=A_.mult, op1=A_.add)
                if debug_taps:
                    nc.sync.dma_start(taps["a2n"][:, blk], a2nb[:])
                nc.vector.tensor_tensor(               # enc = a1 + a2n (in place)
                    a1[:, blk], a1[:, blk], a2nb[:], op=A_.add)
                if debug_taps:
                    nc.sync.dma_start(taps["a1"][:, blk], a1[:, blk])

            # ---------------- m_in LIF scan -> sign spikes ----------------
            Ssg = accp.tile([96, NCH * BT], f16)
            m_in = [stp.tile([96, NCH * BL], f32, tag=f"m_in{i}", name=f"m_in{i}") for i in range(3)]
            x_in = [stp.tile([96, NCH * BL], f32, tag=f"x_in{i}", name=f"x_in{i}") for i in range(2)]
            nc.vector.memset(m_in[0][:], 0.0)
            sgv = Ssg[:].rearrange("p (k b t) -> p k b t", k=NCH, b=BL)
            mv = [m[:].rearrange("p (k b) -> p k b", k=NCH) for m in m_in]
            xv = [x[:].rearrange("p (k b) -> p k b", k=NCH) for x in x_in]
            for t in range(T):
                cur, nxt = t % 3, (t + 1) % 3
                nc.vector.scalar_tensor_tensor(      # x = (m<TH)*m
                    x_in[t % 2][:], m_in[cur][:], TH, m_in[cur][:],
                    op0=A_.is_lt, op1=A_.mult)
                nc.vector.scalar_tensor_tensor(      # m' = x*dm + enc[t]
                    mv[nxt], xv[t % 2], dm, a1v[:, :, :, t],
                    op0=A_.mult, op1=A_.add)
                nc.scalar.sign(sgv[:, :, :, t], mv[nxt], bias=negTH[0:96])

            if debug_taps:
                nc.sync.dma_start(taps["sg"][:], Ssg[:])

            # ---------------- hidden projection matmul -------------------
            phi = mps.tile([BT, HID], f32, tag="phi")
            plo = mps.tile([BT, HID], f32, tag="plo")
            for kk in range(NCH // 2):
                wt = wp.tile([96, 4 * HID], f16, tag="wh")
                nc.sync.dma_start(wt[:], whl_d[kk])
                for s in range(2):
                    k = 2 * kk + s
                    lhs = Ssg[:, k * BT : (k + 1) * BT]
                    nc.tensor.matmul(phi[:], lhs, wt[:, (2 * s) * HID : (2 * s + 1) * HID],
                                     start=(k == 0), stop=(k == NCH - 1))
                    nc.tensor.matmul(plo[:], lhs, wt[:, (2 * s + 1) * HID : (2 * s + 2) * HID],
                                     start=(k == 0), stop=(k == NCH - 1))
            projsb = stp.tile([BT, HID], f32)
            nc.scalar.copy(projsb[:], phi[:])
            nc.vector.scalar_tensor_tensor(          # proj += plo * 2^-11
                projsb[:], plo[:], 1.0 / LO_SC, projsb[:], op0=A_.mult, op1=A_.add)
            if debug_taps:
                nc.sync.dma_start(taps["pj"][:], projsb[:])

            # ---------------- transpose proj; c_h scan --------------------
            projT = stp.tile([128, 4 * BT], f32)
            id120 = ident[0:BT, 0:BT]
            for k in range(4):
                pst = tps.tile([128, BT], f32, tag="tp")
                nc.tensor.transpose(pst[:], projsb[:, k * 128 : (k + 1) * 128], id120)
                nc.scalar.activation(                 # evac + colsum bias
                    projT[:, k * BT : (k + 1) * BT], pst[:],
                    F_.Identity, bias=csh[:, k : k + 1], scale=1.0)
            pds_h = patp.tile([128, 4 * BT], f32)
            seedh = cst.tile([128, T], f32)
            nc.vector.memset(seedh[:, 0:1], 0.0)
            nc.vector.memset(seedh[:, 1:T], ds)
            for s in range(4 * BL):
                nc.scalar.copy(pds_h[:, s * T : (s + 1) * T], seedh[:])
            ch = stp.tile([128, 4 * BT], f32)
            nc.vector.tensor_tensor_scan(
                ch[:], pds_h[:], projT[:], 0.0, op0=A_.mult, op1=A_.add)
            if debug_taps:
                nc.sync.dma_start(taps["ch"][:], ch[:])

            # ---------------- m_h LIF scan -------------------------------
            Ssh = stp.tile([128, 4 * BT], f16)
            m_h = [stp.tile([128, 4 * BL], f32, tag=f"m_h{i}", name=f"m_h{i}") for i in range(3)]
            x_h = stp.tile([128, 4 * BL], f32)
            nc.vector.memset(m_h[0][:], 0.0)
            chv = ch[:].rearrange("p (k b t) -> p k b t", k=4, b=BL)
            shv = Ssh[:].rearrange("p (k b t) -> p k b t", k=4, b=BL)
            mhv = [m[:].rearrange("p (k b) -> p k b", k=4) for m in m_h]
            for t in range(T):
                cur, nxt = t % 3, (t + 1) % 3
                nc.vector.scalar_tensor_tensor(
                    x_h[:], m_h[cur][:], TH, m_h[cur][:], op0=A_.is_lt, op1=A_.mult)
                nc.vector.scalar_tensor_tensor(
                    mhv[nxt], x_h[:].rearrange("p (k b) -> p k b", k=4),
                    dm, chv[:, :, :, t], op0=A_.mult, op1=A_.add)
                nc.scalar.sign(shv[:, :, :, t], mhv[nxt], bias=negTH[:])
            if debug_taps:
                shf = stp.tile([128, 4 * BT], f32)
                nc.vector.tensor_copy(shf[:], Ssh[:])
                nc.sync.dma_start(taps["sh"][:], shf[:])

            # ---------------- output projection --------------------------
            pho = tps.tile([BT, OUT], f32, tag="po")
            plo2 = tps.tile([BT, OUT], f32, tag="po")
            for k in range(4):
                lhs = Ssh[:, k * BT : (k + 1) * BT]
                nc.tensor.matmul(pho[:], lhs, wohi[:, k * OUT : (k + 1) * OUT],
                                 start=(k == 0), stop=(k == 3))
                nc.tensor.matmul(plo2[:], lhs, wolo[:, k * OUT : (k + 1) * OUT],
                                 start=(k == 0), stop=(k == 3))
            posb = stp.tile([BT, OUT], f32)
            nc.scalar.copy(posb[:], pho[:])
            nc.vector.scalar_tensor_tensor(
                posb[:], plo2[:], 1.0 / LO_SC, posb[:], op0=A_.mult, op1=A_.add)
            pot = tps.tile([OUT, BT], f32, tag="tp")
            nc.tensor.transpose(pot[:], posb[:], id120)
            poT = stp.tile([OUT, BT], f32)
            nc.scalar.activation(poT[:], pot[:], F_.Identity, bias=cso[:], scale=1.0)
            if debug_taps:
                nc.sync.dma_start(taps["po"][:], poT[:])

            pds_o = patp.tile([OUT, BT], f32)
            seedo = cst.tile([OUT, T], f32)
            nc.vector.memset(seedo[:, 0:1], 0.0)
            nc.vector.memset(seedo[:, 1:T], ds)
            for s in range(BL):
                nc.scalar.copy(pds_o[:, s * T : (s + 1) * T], seedo[:])
            co = stp.tile([OUT, BT], f32)
            nc.vector.tensor_tensor_scan(
                co[:], pds_o[:], poT[:], 0.0, op0=A_.mult, op1=A_.add)

            # ---------------- m_o LIF scan + spike count ------------------
            Sso = stp.tile([OUT, BT], f32)
            m_o = [stp.tile([OUT, BL], f32, tag=f"m_o{i}", name=f"m_o{i}") for i in range(3)]
            x_o = stp.tile([OUT, BL], f32)
            nc.vector.memset(m_o[0][:], 0.0)
            cov = co[:].rearrange("p (b t) -> p b t", b=BL)
            sov = Sso[:].rearrange("p (b t) -> p b t", b=BL)
            for t in range(T):
                cur, nxt = t % 3, (t + 1) % 3
                nc.vector.scalar_tensor_tensor(
                    x_o[:], m_o[cur][:], TH, m_o[cur][:], op0=A_.is_lt, op1=A_.mult)
                nc.vector.scalar_tensor_tensor(
                    m_o[nxt][:], x_o[:], dm, cov[:, :, t], op0=A_.mult, op1=A_.add)
                nc.scalar.sign(sov[:, :, t], m_o[nxt][:], bias=negTH[0:OUT])
            # out = (sum_t sgn + T) / (2T)
            accT = stp.tile([OUT, BL], f32)
            nc.vector.tensor_reduce(
                accT[:], Sso[:].rearrange("p (b t) -> p b t", b=BL),
                axis=mybir.AxisListType.X, op=A_.add)
            outsb = stp.tile([OUT, BL], f32)
            nc.vector.tensor_scalar(
                outsb[:], accT[:], 1.0 / (2.0 * T), 0.5, op0=A_.mult, op1=A_.add)
            nc.sync.dma_start(out_d[:].rearrange("b o -> o b"), outsb[:])

    nc.finalize()
    return nc


def _host_prep(events, w_conv, w_hid, w_out):
    ev = np.clip(events, 0, 1).astype(np.float16)          # [B,T,2,H,W] {0,1}
    # residue-shuffled, h-major, margin-padded: evr[h,b,t,c,r,1+j] = ev[b,t,c,h,4j+r]
    evr = np.zeros((128, B, T, 2, 4, 34), np.float16)
    evr[:, :, :, :, :, 1:33] = (
        ev.reshape(B, T, 2, H, 32, 4).transpose(3, 0, 1, 2, 5, 4))

    wc = (np.asarray(w_conv, np.float64) * SCALE * CONV_SC).astype(np.float32)
    A = np.zeros((8, 128, 96), np.float32)
    for c in range(2):
        for q in range(4):
            for di in range(3):
                for i in range(32):
                    for p in range(4):
                        h = 4 * i + di - 1 + p
                        if 0 <= h < 128:
                            A[c * 4 + q, h, di * 32 + i] = wc[c, p, q]
    ahi = A.astype(np.float16)
    alo = (A - ahi.astype(np.float32)).astype(np.float16)

    Wh = (np.asarray(w_hid, np.float64) / 2.0).astype(np.float32)   # [512, 9216]
    WT = np.ascontiguousarray(Wh.T)                                  # [9216, 512]
    djs, js = np.divmod(np.arange(NCH), 32)
    dis, is_ = np.divmod(np.arange(96), 32)
    fmap = ((dis[None, :] * 3 + djs[:, None]) * 1024
            + is_[None, :] * 32 + js[:, None])                       # [NCH, 96]
    whP = WT[fmap]                                                   # [NCH, 96, 512]
    whi = whP.astype(np.float16)
    wlo = ((whP - whi.astype(np.float32)) * LO_SC).astype(np.float16)
    # interleave (hi, lo) per chunk-pair: [NCH//2, 96, 4*HID]
    whl = np.empty((NCH // 2, 96, 4 * HID), np.float16)
    for s in range(2):
        whl[:, :, 2 * s * HID : (2 * s + 1) * HID] = whi[s::2].transpose(0, 1, 2)[
            np.arange(NCH // 2)] if False else whi[2 * np.arange(NCH // 2) + s]
        whl[:, :, (2 * s + 1) * HID : (2 * s + 2) * HID] = wlo[2 * np.arange(NCH // 2) + s]
    csh = Wh.sum(axis=1).astype(np.float32).reshape(4, 128)

    WoC = (np.asarray(w_out, np.float64).T / 2.0).astype(np.float32).reshape(4, 128, OUT)
    wohi = WoC.astype(np.float16)
    wolo = ((WoC - wohi.astype(np.float32)) * LO_SC).astype(np.float16)
    cso = (np.asarray(w_out, np.float64).sum(axis=1) / 2.0).astype(np.float32)

    shared = dict(ahi=ahi, alo=alo, whl=whl, csh=csh, wohi=wohi, wolo=wolo,
                  cso=cso, ident=np.eye(128, dtype=np.float32))
    return evr, shared


def _run(events, w_conv, w_hid, w_out, debug_taps=False, trace=False, ncores=NCORES):
    key = ("nc", debug_taps)
    if key not in _CACHE:
        _CACHE[key] = _build_program(debug_taps=debug_taps)
    nc = _CACHE[key]
    ev, shared = _host_prep(events, w_conv, w_hid, w_out)
    in_maps = []
    for c in range(ncores):
        m = {"ev": np.ascontiguousarray(ev[:, c * BL : (c + 1) * BL])}
        m.update(shared)
        in_maps.append(m)
    res = run_bass_kernel_spmd(nc, in_maps, core_ids=list(range(ncores)), trace=trace)
    out = np.concatenate([res.results[c]["out"] for c in range(ncores)], axis=0)
    return out.astype(np.float32), res


def kernel(events, w_conv, w_hid, w_out, batch_size):
    out, _ = _run(np.asarray(events), np.asarray(w_conv),
                  np.asarray(w_hid), np.asarray(w_out))
    return out



# revision 11
# speedup vs baseline: 1.0449x; 1.0449x over previous
"""DTS-SNN 2D Trainium2 kernel (8 NeuronCores, batch-data-parallel).

Reference math:
  e in {0,1}; tr1/tr2 leaky traces; enc = blockconv4x4(unfold3x3((tr1-tr2)*SCALE))
  m_in LIF -> s_in; c_h = c_h*ds + s_in@Wh.T; m_h LIF -> s_h;
  c_o = c_o*ds + s_h@Wo.T; m_o LIF -> s_o; out = sum_t(s_o)/T

Hardware restructuring (exactness ~1e-7, required: SNN flips spikes at ~1e-5):
  EC[t] = conv(e[t])  on PE as 16 matmuls/group: fp16 hi + fp16 lo stationaries
          (both scaled 2^6; evacuation copy applies 2^-6), moving = binary
          events (exact fp16), contraction over H, band-clipped stationaries.
  a1/a2 = per-feature leaky scans of EC via tensor_tensor_scan, segmented by a
          decay pattern with 0 at segment starts; a2 computed negated
          (op1=subtract) so enc[t] = a1[t] + a2n[t].
  m_in scan (t-sequential, DVE): x=(m<TH)*m; m'=x*dm+a1[t]; m'+=a2n[t];
          spike via ACT Sign(m'-TH) in {-1,+1} stored fp16.
  proj = Ssgn @ (Wh.T/2) + colsum(Wh)/2  -- one big matmul over all (b,t),
          fp16 hi/lo split (lo scaled 2^11, separate PSUM bank, combined once).
  c_h/c_o via tensor_tensor_scan after a PE transpose (csum bias folded into
          the PSUM-evacuation activation); m_h/m_o scans like m_in.

Sharding: batch 16 -> 2 per core; all weights replicated (hint-compliant).
"""

import numpy as np

import concourse.bacc as bacc
import concourse.mybir as mybir
import concourse.tile as tile
from concourse.bass_utils import run_bass_kernel_spmd

# ---- model constants -------------------------------------------------------
B, T, H, W = 16, 60, 128, 128
NCORES = 8
BL = B // NCORES
HID, OUT = 512, 11
NCH = 96                    # feature chunks, one per (dj, j); 96 feats each
TH = 0.3
SCALE = 0.5
d1 = float(np.exp(-1.0 / 20.0))
d2 = float(np.exp(-1.0 / 5.0))
dm = float(np.exp(-1.0 / 20.0))
ds = float(np.exp(-1.0 / 5.0))
CONV_SC = 2.0 ** 6          # conv stationaries pre-scaled; evac applies 2^-6
LO_SC = 2.0 ** 11           # lo-residual scale for the hidden/output weights
FW = 136                    # frame: 4 residue planes x 34 (margins baked on host)
NFR = BL * T * 2            # 240 image planes per core
FGRP = 15                   # tau frames per conv psum group
NFG = T // FGRP
FB = 6                      # feature blocks = (dj, jhalf)
CHB = NCH // FB             # 16 chunks per feature block
BT = BL * T
WPREF = 16                  # rolling weight-prefetch depth (slots of 4KiB/par)
f16 = mybir.dt.float16
f32 = mybir.dt.float32
A_ = mybir.AluOpType
F_ = mybir.ActivationFunctionType

_CACHE: dict = {}


def _build_program(debug_taps=False):
    nc = bacc.Bacc("TRN2", target_bir_lowering=False, debug=True)

    ev_d = nc.dram_tensor("ev", [128, BL, T, 2, 4, 34], f16, kind="ExternalInput")
    ahi_d = nc.dram_tensor("ahi", [8, 128, 96], f16, kind="ExternalInput")
    alo_d = nc.dram_tensor("alo", [8, 128, 96], f16, kind="ExternalInput")
    whl_d = nc.dram_tensor("whl", [NCH // 2, 96, 4 * HID], f16, kind="ExternalInput")
    csh_d = nc.dram_tensor("csh", [4, 128], f32, kind="ExternalInput")
    wohi_d = nc.dram_tensor("wohi", [4, 128, OUT], f16, kind="ExternalInput")
    wolo_d = nc.dram_tensor("wolo", [4, 128, OUT], f16, kind="ExternalInput")
    cso_d = nc.dram_tensor("cso", [OUT], f32, kind="ExternalInput")
    id_d = nc.dram_tensor("ident", [128, 128], f32, kind="ExternalInput")
    p1_d = nc.dram_tensor("p1", [96, CHB * BL * T], f32, kind="ExternalInput")
    p2_d = nc.dram_tensor("p2", [96, CHB * BL * T], f32, kind="ExternalInput")
    pdh_d = nc.dram_tensor("pdh", [128, 4 * BT], f32, kind="ExternalInput")
    pdo_d = nc.dram_tensor("pdo", [OUT, BT], f32, kind="ExternalInput")
    out_d = nc.dram_tensor("out", [BL, OUT], f32, kind="ExternalOutput")
    taps = {}
    if debug_taps:
        for nm, shp in [("ec", [96, NCH * BT]), ("a1", [96, NCH * BT]),
                        ("a2n", [96, NCH * BT]),
                        ("pj", [BT, HID]), ("ch", [128, 4 * BT]),
                        ("sh", [128, 4 * BT]), ("po", [OUT, BT])]:
            taps[nm] = nc.dram_tensor("tap_" + nm, shp, f32, kind="ExternalOutput")
        taps["sg"] = nc.dram_tensor("tap_sg", [96, NCH * BT], f16, kind="ExternalOutput")

    with tile.TileContext(nc) as tc:
        with (
            tc.tile_pool(name="const", bufs=1) as cst,
            tc.tile_pool(name="acc", bufs=1) as accp,
            tc.tile_pool(name="pat", bufs=1) as patp,
            tc.tile_pool(name="state", bufs=1) as stp,
            tc.tile_pool(name="cpsum", bufs=2, space="PSUM") as cps,
            tc.tile_pool(name="mpsum", bufs=1, space="PSUM") as mps,
            tc.tile_pool(name="tpsum", bufs=2, space="PSUM") as tps,
        ):
            # ---------------- constants / weights in SBUF ----------------
            ahi = cst.tile([128, 8 * 96], f16)
            alo = cst.tile([128, 8 * 96], f16)
            nc.sync.dma_start(ahi[:].rearrange("p (k m) -> p k m", k=8),
                              ahi_d[:].rearrange("k p m -> p k m"))
            nc.sync.dma_start(alo[:].rearrange("p (k m) -> p k m", k=8),
                              alo_d[:].rearrange("k p m -> p k m"))
            csh = cst.tile([128, 4], f32)
            nc.sync.dma_start(csh[:], csh_d[:].rearrange("k p -> p k"))
            wohi = cst.tile([128, 4 * OUT], f16)
            wolo = cst.tile([128, 4 * OUT], f16)
            nc.sync.dma_start(wohi[:].rearrange("p (k m) -> p k m", k=4),
                              wohi_d[:].rearrange("k p m -> p k m"))
            nc.sync.dma_start(wolo[:].rearrange("p (k m) -> p k m", k=4),
                              wolo_d[:].rearrange("k p m -> p k m"))
            cso = cst.tile([OUT, 1], f32)
            nc.sync.dma_start(cso[:], cso_d[:].rearrange("(p o) -> p o", o=1))
            ident = cst.tile([128, 128], f32)
            nc.sync.dma_start(ident[:], id_d[:])
            negTH = cst.tile([128, 1], f32)
            nc.vector.memset(negTH[:], -TH)

            # ---------------- filter patterns (host-precomputed) ----------
            p1 = patp.tile([96, CHB * BL * T], f32)
            p2 = patp.tile([96, CHB * BL * T], f32)
            pds_h = patp.tile([128, 4 * BT], f32)
            pds_o = patp.tile([OUT, BT], f32)
            nc.sync.dma_start(p1[:], p1_d[:])
            nc.sync.dma_start(p2[:], p2_d[:])
            nc.sync.dma_start(pds_h[:], pdh_d[:])
            nc.sync.dma_start(pds_o[:], pdo_d[:])

            # ---------------- conv + temporal filters --------------------
            # a1 free layout: (chunk=(dj,jh,j) 96, b 2, t 60)
            a1 = accp.tile([96, NCH * BT], f32)
            a1v = a1[:].rearrange("p (k b t) -> p k b t", k=NCH, b=BL)
            BLKF = CHB * BL * T
            with tc.tile_pool(name="ev", bufs=1) as evp:
                # events -> SBUF (h on partitions)
                EV = evp.tile([128, NFR * FW], f16)
                ev3 = EV[:].rearrange("p (f w) -> p f w", f=NFR)
                ev6 = EV[:].rearrange("p (b t c r j) -> p b t c r j",
                                      b=BL, t=T, c=2, r=4)
                src = ev_d[:].rearrange("p b t c r j -> p (b t c) (r j)")
                NDMA = 8
                FPD = NFR // NDMA
                for k in range(NDMA):
                    nc.sync.dma_start(
                        ev3[:, k * FPD : (k + 1) * FPD, :],
                        src[:, k * FPD : (k + 1) * FPD, :],
                    )
                for dj in range(3):
                    for b in range(BL):
                        for g in range(NFG):
                            ps = cps.tile([96, FGRP * 32], f32)
                            t0 = g * FGRP
                            first = True
                            for c in range(2):
                                for q in range(4):
                                    r0 = (q + dj - 1) % 4
                                    j0 = (q + dj - 1) // 4
                                    rhs = ev6[:, b, t0 : t0 + FGRP, c, r0,
                                              j0 + 1 : j0 + 33]
                                    for wt in (ahi, alo):
                                        nc.tensor.matmul(
                                            ps[:],
                                            wt[:, (c * 4 + q) * 96 : (c * 4 + q + 1) * 96],
                                            rhs,
                                            start=first,
                                            stop=(c == 1 and q == 3 and wt is alo),
                                        )
                                        first = False
                            dst = a1v[:, dj * 32 : dj * 32 + 32, b, t0 : t0 + FGRP]
                            srcp = ps[:].rearrange("p (t j) -> p j t", t=FGRP)
                            nc.scalar.activation(dst, srcp, F_.Copy,
                                                 scale=1.0 / CONV_SC)
            # EV pool released: the weight-stream pool below reuses its SBUF.
            wp = tc.alloc_tile_pool(name="w", bufs=WPREF)
            wt_tiles = []
            for kk in range(NCH // 2):
                wt = wp.tile([96, 4 * HID], f16, tag="wh")
                nc.sync.dma_start(wt[:], whl_d[kk])
                wt_tiles.append(wt)
            for fb in range(FB):
                blk = slice(fb * BLKF, (fb + 1) * BLKF)
                if debug_taps:
                    nc.sync.dma_start(taps["ec"][:, blk], a1[:, blk])
                a2nb = accp.tile([96, BLKF], f32, tag="a2nb", bufs=2, name="a2nb")
                nc.vector.tensor_tensor_scan(          # a2nb = -a2
                    a2nb[:], p2[:], a1[:, blk], 0.0, op0=A_.mult, op1=A_.subtract)
                nc.vector.tensor_tensor_scan(          # a1 (in place over EC)
                    a1[:, blk], p1[:], a1[:, blk], 0.0, op0=A_.mult, op1=A_.add)
                if debug_taps:
                    nc.sync.dma_start(taps["a2n"][:, blk], a2nb[:])
                nc.vector.tensor_tensor(               # enc = a1 + a2n (in place)
                    a1[:, blk], a1[:, blk], a2nb[:], op=A_.add)
                if debug_taps:
                    nc.sync.dma_start(taps["a1"][:, blk], a1[:, blk])

            # ---------------- m_in LIF scan -> sign spikes ----------------
            Ssg = accp.tile([96, NCH * BT], f16)
            m_in = [stp.tile([96, NCH * BL], f32, tag=f"m_in{i}", name=f"m_in{i}") for i in range(3)]
            x_in = [stp.tile([96, NCH * BL], f32, tag=f"x_in{i}", name=f"x_in{i}") for i in range(2)]
            nc.vector.memset(m_in[0][:], 0.0)
            sgv = Ssg[:].rearrange("p (k b t) -> p k b t", k=NCH, b=BL)
            mv = [m[:].rearrange("p (k b) -> p k b", k=NCH) for m in m_in]
            xv = [x[:].rearrange("p (k b) -> p k b", k=NCH) for x in x_in]
            for t in range(T):
                cur, nxt = t % 3, (t + 1) % 3
                nc.vector.scalar_tensor_tensor(      # x = (m<TH)*m
                    x_in[t % 2][:], m_in[cur][:], TH, m_in[cur][:],
                    op0=A_.is_lt, op1=A_.mult)
                nc.vector.scalar_tensor_tensor(      # m' = x*dm + enc[t]
                    mv[nxt], xv[t % 2], dm, a1v[:, :, :, t],
                    op0=A_.mult, op1=A_.add)
                nc.scalar.sign(sgv[:, :, :, t], mv[nxt], bias=negTH[0:96])

            if debug_taps:
                nc.sync.dma_start(taps["sg"][:], Ssg[:])

            # ---------------- hidden projection matmul -------------------
            phi = mps.tile([BT, HID], f32, tag="phi")
            plo = mps.tile([BT, HID], f32, tag="plo")
            for kk in range(NCH // 2):
                wt = wt_tiles[kk]
                for s in range(2):
                    k = 2 * kk + s
                    lhs = Ssg[:, k * BT : (k + 1) * BT]
                    nc.tensor.matmul(phi[:], lhs, wt[:, (2 * s) * HID : (2 * s + 1) * HID],
                                     start=(k == 0), stop=(k == NCH - 1))
                    nc.tensor.matmul(plo[:], lhs, wt[:, (2 * s + 1) * HID : (2 * s + 2) * HID],
                                     start=(k == 0), stop=(k == NCH - 1))
            wp.release()
            projsb = stp.tile([BT, HID], f32)
            nc.scalar.copy(projsb[:], phi[:])
            nc.vector.scalar_tensor_tensor(          # proj += plo * 2^-11
                projsb[:], plo[:], 1.0 / LO_SC, projsb[:], op0=A_.mult, op1=A_.add)
            if debug_taps:
                nc.sync.dma_start(taps["pj"][:], projsb[:])

            # ---------------- transpose proj; c_h scan --------------------
            projT = stp.tile([128, 4 * BT], f32)
            id120 = ident[0:BT, 0:BT]
            for k in range(4):
                pst = tps.tile([128, BT], f32, tag="tp")
                nc.tensor.transpose(pst[:], projsb[:, k * 128 : (k + 1) * 128], id120)
                nc.scalar.activation(                 # evac + colsum bias
                    projT[:, k * BT : (k + 1) * BT], pst[:],
                    F_.Identity, bias=csh[:, k : k + 1], scale=1.0)
            ch = stp.tile([128, 4 * BT], f32)
            nc.vector.tensor_tensor_scan(
                ch[:], pds_h[:], projT[:], 0.0, op0=A_.mult, op1=A_.add)
            if debug_taps:
                nc.sync.dma_start(taps["ch"][:], ch[:])

            # ---------------- m_h LIF scan -------------------------------
            NR = 8
            Ssh = stp.tile([128, 4 * BT], f16)
            m_h = [stp.tile([128, 4 * BL], f32, tag=f"m_h{i}", name=f"m_h{i}") for i in range(NR)]
            x_h = stp.tile([128, 4 * BL], f32)
            nc.vector.memset(m_h[0][:], 0.0)
            chv = ch[:].rearrange("p (k b t) -> p k b t", k=4, b=BL)
            shv = Ssh[:].rearrange("p (k b t) -> p k b t", k=4, b=BL)
            mhv = [m[:].rearrange("p (k b) -> p k b", k=4) for m in m_h]
            for t in range(T):
                cur, nxt = t % NR, (t + 1) % NR
                nc.vector.scalar_tensor_tensor(
                    x_h[:], m_h[cur][:], TH, m_h[cur][:], op0=A_.is_lt, op1=A_.mult)
                nc.vector.scalar_tensor_tensor(
                    mhv[nxt], x_h[:].rearrange("p (k b) -> p k b", k=4),
                    dm, chv[:, :, :, t], op0=A_.mult, op1=A_.add)
                nc.scalar.sign(shv[:, :, :, t], mhv[nxt], bias=negTH[:])
            if debug_taps:
                shf = stp.tile([128, 4 * BT], f32)
                nc.vector.tensor_copy(shf[:], Ssh[:])
                nc.sync.dma_start(taps["sh"][:], shf[:])

            # ---------------- output projection --------------------------
            pho = tps.tile([BT, OUT], f32, tag="po")
            plo2 = tps.tile([BT, OUT], f32, tag="po")
            for k in range(4):
                lhs = Ssh[:, k * BT : (k + 1) * BT]
                nc.tensor.matmul(pho[:], lhs, wohi[:, k * OUT : (k + 1) * OUT],
                                 start=(k == 0), stop=(k == 3))
                nc.tensor.matmul(plo2[:], lhs, wolo[:, k * OUT : (k + 1) * OUT],
                                 start=(k == 0), stop=(k == 3))
            posb = stp.tile([BT, OUT], f32)
            nc.scalar.copy(posb[:], pho[:])
            nc.vector.scalar_tensor_tensor(
                posb[:], plo2[:], 1.0 / LO_SC, posb[:], op0=A_.mult, op1=A_.add)
            pot = tps.tile([OUT, BT], f32, tag="tp")
            nc.tensor.transpose(pot[:], posb[:], id120)
            poT = stp.tile([OUT, BT], f32)
            nc.scalar.activation(poT[:], pot[:], F_.Identity, bias=cso[:], scale=1.0)
            if debug_taps:
                nc.sync.dma_start(taps["po"][:], poT[:])

            co = stp.tile([OUT, BT], f32)
            nc.vector.tensor_tensor_scan(
                co[:], pds_o[:], poT[:], 0.0, op0=A_.mult, op1=A_.add)

            # ---------------- m_o LIF scan + spike count ------------------
            Sso = stp.tile([OUT, BT], f32)
            m_o = [stp.tile([OUT, BL], f32, tag=f"m_o{i}", name=f"m_o{i}") for i in range(NR)]
            x_o = stp.tile([OUT, BL], f32)
            nc.vector.memset(m_o[0][:], 0.0)
            cov = co[:].rearrange("p (b t) -> p b t", b=BL)
            sov = Sso[:].rearrange("p (b t) -> p b t", b=BL)
            for t in range(T):
                cur, nxt = t % NR, (t + 1) % NR
                nc.vector.scalar_tensor_tensor(
                    x_o[:], m_o[cur][:], TH, m_o[cur][:], op0=A_.is_lt, op1=A_.mult)
                nc.vector.scalar_tensor_tensor(
                    m_o[nxt][:], x_o[:], dm, cov[:, :, t], op0=A_.mult, op1=A_.add)
                nc.scalar.sign(sov[:, :, t], m_o[nxt][:], bias=negTH[0:OUT])
            # out = (sum_t sgn + T) / (2T)
            accT = stp.tile([OUT, BL], f32)
            nc.vector.tensor_reduce(
                accT[:], Sso[:].rearrange("p (b t) -> p b t", b=BL),
                axis=mybir.AxisListType.X, op=A_.add)
            outsb = stp.tile([OUT, BL], f32)
            nc.vector.tensor_scalar(
                outsb[:], accT[:], 1.0 / (2.0 * T), 0.5, op0=A_.mult, op1=A_.add)
            nc.sync.dma_start(out_d[:].rearrange("b o -> o b"), outsb[:])

    nc.finalize()
    return nc


def _host_prep(events, w_conv, w_hid, w_out):
    ev = np.clip(events, 0, 1).astype(np.float16)          # [B,T,2,H,W] {0,1}
    # residue-shuffled, h-major, margin-padded: evr[h,b,t,c,r,1+j] = ev[b,t,c,h,4j+r]
    evr = np.zeros((128, B, T, 2, 4, 34), np.float16)
    evr[:, :, :, :, :, 1:33] = (
        ev.reshape(B, T, 2, H, 32, 4).transpose(3, 0, 1, 2, 5, 4))

    wc = (np.asarray(w_conv, np.float64) * SCALE * CONV_SC).astype(np.float32)
    A = np.zeros((8, 128, 96), np.float32)
    for c in range(2):
        for q in range(4):
            for di in range(3):
                for i in range(32):
                    for p in range(4):
                        h = 4 * i + di - 1 + p
                        if 0 <= h < 128:
                            A[c * 4 + q, h, di * 32 + i] = wc[c, p, q]
    ahi = A.astype(np.float16)
    alo = (A - ahi.astype(np.float32)).astype(np.float16)

    Wh = (np.asarray(w_hid, np.float64) / 2.0).astype(np.float32)   # [512, 9216]
    WT = np.ascontiguousarray(Wh.T)                                  # [9216, 512]
    djs, js = np.divmod(np.arange(NCH), 32)
    dis, is_ = np.divmod(np.arange(96), 32)
    fmap = ((dis[None, :] * 3 + djs[:, None]) * 1024
            + is_[None, :] * 32 + js[:, None])                       # [NCH, 96]
    whP = WT[fmap]                                                   # [NCH, 96, 512]
    whi = whP.astype(np.float16)
    wlo = ((whP - whi.astype(np.float32)) * LO_SC).astype(np.float16)
    # interleave (hi, lo) per chunk-pair: [NCH//2, 96, 4*HID]
    whl = np.empty((NCH // 2, 96, 4 * HID), np.float16)
    for s in range(2):
        whl[:, :, 2 * s * HID : (2 * s + 1) * HID] = whi[s::2].transpose(0, 1, 2)[
            np.arange(NCH // 2)] if False else whi[2 * np.arange(NCH // 2) + s]
        whl[:, :, (2 * s + 1) * HID : (2 * s + 2) * HID] = wlo[2 * np.arange(NCH // 2) + s]
    csh = Wh.sum(axis=1).astype(np.float32).reshape(4, 128)

    WoC = (np.asarray(w_out, np.float64).T / 2.0).astype(np.float32).reshape(4, 128, OUT)
    wohi = WoC.astype(np.float16)
    wolo = ((WoC - wohi.astype(np.float32)) * LO_SC).astype(np.float16)
    cso = (np.asarray(w_out, np.float64).sum(axis=1) / 2.0).astype(np.float32)

    def seg_pattern(nrow, nseg, decay):
        row = np.full(T, decay, np.float32)
        row[0] = 0.0
        return np.broadcast_to(np.tile(row, nseg), (nrow, nseg * T)).copy()

    shared = dict(ahi=ahi, alo=alo, whl=whl, csh=csh, wohi=wohi, wolo=wolo,
                  cso=cso, ident=np.eye(128, dtype=np.float32),
                  p1=seg_pattern(96, CHB * BL, d1),
                  p2=seg_pattern(96, CHB * BL, d2),
                  pdh=seg_pattern(128, 4 * BL, ds),
                  pdo=seg_pattern(OUT, BL, ds))
    return evr, shared


def _run(events, w_conv, w_hid, w_out, debug_taps=False, trace=False, ncores=NCORES):
    key = ("nc", debug_taps)
    if key not in _CACHE:
        _CACHE[key] = _build_program(debug_taps=debug_taps)
    nc = _CACHE[key]
    ev, shared = _host_prep(events, w_conv, w_hid, w_out)
    in_maps = []
    for c in range(ncores):
        m = {"ev": np.ascontiguousarray(ev[:, c * BL : (c + 1) * BL])}
        m.update(shared)
        in_maps.append(m)
    res = run_bass_kernel_spmd(nc, in_maps, core_ids=list(range(ncores)), trace=trace)
    out = np.concatenate([res.results[c]["out"] for c in range(ncores)], axis=0)
    return out.astype(np.float32), res


def kernel(events, w_conv, w_hid, w_out, batch_size):
    out, _ = _run(np.asarray(events), np.asarray(w_conv),
                  np.asarray(w_hid), np.asarray(w_out))
    return out



# revision 17
# speedup vs baseline: 1.1995x; 1.1479x over previous
"""DTS-SNN 2D Trainium2 kernel (8 NeuronCores, batch-data-parallel).

Reference math:
  e in {0,1}; tr1/tr2 leaky traces; enc = blockconv4x4(unfold3x3((tr1-tr2)*SCALE))
  m_in LIF -> s_in; c_h = c_h*ds + s_in@Wh.T; m_h LIF -> s_h;
  c_o = c_o*ds + s_h@Wo.T; m_o LIF -> s_o; out = sum_t(s_o)/T

Hardware restructuring (exactness ~1e-7, required: SNN flips spikes at ~1e-5):
  EC[t] = conv(e[t])  on PE as 16 matmuls/group: fp16 hi + fp16 lo stationaries
          (both scaled 2^6; evacuation copy applies 2^-6), moving = binary
          events (exact fp16), contraction over H, band-clipped stationaries.
  a1/a2 = per-feature leaky scans of EC via tensor_tensor_scan, segmented by a
          decay pattern with 0 at segment starts; a2 computed negated
          (op1=subtract) so enc[t] = a1[t] + a2n[t].
  m_in scan (t-sequential, DVE): x=(m<TH)*m; m'=x*dm+a1[t]; m'+=a2n[t];
          spike via ACT Sign(m'-TH) in {-1,+1} stored fp16.
  proj = Ssgn @ (Wh.T/2) + colsum(Wh)/2  -- one big matmul over all (b,t),
          fp16 hi/lo split (lo scaled 2^11, separate PSUM bank, combined once).
  c_h/c_o via tensor_tensor_scan after a PE transpose (csum bias folded into
          the PSUM-evacuation activation); m_h/m_o scans like m_in.

Sharding: batch 16 -> 2 per core; all weights replicated (hint-compliant).
"""

import numpy as np

import concourse.bacc as bacc
import concourse.mybir as mybir
import concourse.tile as tile
from concourse.bass_utils import run_bass_kernel_spmd

import concourse.dve_ops as _dve_ops
from concourse.dve_spec import (
    Spec as _Spec, Src0 as _S0, Src1 as _S1, C0 as _DC0, C1 as _DC1,
    Zero as _DZ, select as _dsel, lower as _dlower, _has_src1 as _dhas1)
from concourse.dve_uop import DveOpSpec as _DveOpSpec


def _get_lif_op():
    """Fused LIF membrane step as one DVE op:
    out = (in0 < s0 ? in0 : 0) * s1 + in1  ==  m' = (m<TH)*m*dm + enc.
    Registered through the per-NEFF custom-DVE table machinery (same path
    as the stock ops in dve_ops.OPS, just added at runtime)."""
    name = "LIF_STEP_ANT"
    for op in _dve_ops.OPS:
        if op.name == name:
            return op
    spec = _Spec(
        body=_dsel(_S0 < _DC0, _S0, _DZ) * _DC1 + _S1,
        reference=lambda in0, in1, s0, s1, imm2: (
            np.where(in0 < s0, in0, 0.0).astype(np.float32)
            * np.float32(s1) + in1).astype(np.float32),
    )
    row = max(_dve_ops._SUB_OPCODE_FOR_NAME.values()) + 1
    assert row < 0x20
    shas = {
        ver: _DveOpSpec(name=name, opcode=row, uops=_dlower(spec, ver=ver),
                        rd1_en=_dhas1(spec)).sha(ver)
        for ver in ("v3", "v4")
    }
    _dve_ops._SUB_OPCODE_FOR_NAME[name] = row
    op = _dve_ops.DveOp(name, spec, subdim=False, uops_sha=shas)
    _dve_ops.OPS.append(op)
    _dve_ops.CUSTOM_DVE_SPECS[name] = spec
    return op


LIF_OP = _get_lif_op()
USE_LIF = True

# ---- model constants -------------------------------------------------------
B, T, H, W = 16, 60, 128, 128
NCORES = 8
BL = B // NCORES
HID, OUT = 512, 11
NCH = 96                    # feature chunks, one per (dj, j); 96 feats each
TH = 0.3
SCALE = 0.5
d1 = float(np.exp(-1.0 / 20.0))
d2 = float(np.exp(-1.0 / 5.0))
dm = float(np.exp(-1.0 / 20.0))
ds = float(np.exp(-1.0 / 5.0))
CONV_SC = 2.0 ** 6          # conv stationaries pre-scaled; evac applies 2^-6
LO_SC = 2.0 ** 11           # lo-residual scale for the hidden/output weights
FW = 136                    # frame: 4 residue planes x 34 (margins baked on host)
NFR = BL * T * 2            # 240 image planes per core
FGRP = 15                   # tau frames per conv psum group
NFG = T // FGRP
FB = 6                      # feature blocks = (dj, jhalf)
CHB = NCH // FB             # 16 chunks per feature block
BT = BL * T
WPREF = 16                  # rolling weight-prefetch depth (slots of 4KiB/par)
f16 = mybir.dt.float16
f32 = mybir.dt.float32
A_ = mybir.AluOpType
F_ = mybir.ActivationFunctionType

_CACHE: dict = {}


def _build_program(debug_taps=False):
    nc = bacc.Bacc("TRN2", target_bir_lowering=False, debug=True)

    ev_d = nc.dram_tensor("ev", [128, BL, T, 2, 4, 34], f16, kind="ExternalInput")
    ahi_d = nc.dram_tensor("ahi", [8, 128, 96], f16, kind="ExternalInput")
    alo_d = nc.dram_tensor("alo", [8, 128, 96], f16, kind="ExternalInput")
    whl_d = nc.dram_tensor("whl", [NCH // 2, 96, 4 * HID], f16, kind="ExternalInput")
    csh_d = nc.dram_tensor("csh", [4, 128], f32, kind="ExternalInput")
    wohi_d = nc.dram_tensor("wohi", [4, 128, OUT], f16, kind="ExternalInput")
    wolo_d = nc.dram_tensor("wolo", [4, 128, OUT], f16, kind="ExternalInput")
    cso_d = nc.dram_tensor("cso", [OUT], f32, kind="ExternalInput")
    id_d = nc.dram_tensor("ident", [128, 128], f32, kind="ExternalInput")
    p1_d = nc.dram_tensor("p1", [96, CHB * BL * T], f32, kind="ExternalInput")
    p2_d = nc.dram_tensor("p2", [96, CHB * BL * T], f32, kind="ExternalInput")
    pdh_d = nc.dram_tensor("pdh", [128, 4 * BT], f32, kind="ExternalInput")
    pdo_d = nc.dram_tensor("pdo", [OUT, BT], f32, kind="ExternalInput")
    out_d = nc.dram_tensor("out", [BL, OUT], f32, kind="ExternalOutput")
    taps = {}
    if debug_taps:
        for nm, shp in [("ec", [96, NCH * BT]), ("a1", [96, NCH * BT]),
                        ("a2n", [96, NCH * BT]),
                        ("pj", [BT, HID]), ("ch", [128, 4 * BT]),
                        ("sh", [128, 4 * BT]), ("po", [OUT, BT])]:
            taps[nm] = nc.dram_tensor("tap_" + nm, shp, f32, kind="ExternalOutput")
        taps["sg"] = nc.dram_tensor("tap_sg", [96, NCH * BT], f16, kind="ExternalOutput")

    with tile.TileContext(nc) as tc:
        with (
            tc.tile_pool(name="const", bufs=1) as cst,
            tc.tile_pool(name="acc", bufs=1) as accp,
            tc.tile_pool(name="pat", bufs=1) as patp,
            tc.tile_pool(name="state", bufs=1) as stp,
            tc.tile_pool(name="cpsum", bufs=2, space="PSUM") as cps,
            tc.tile_pool(name="mpsum", bufs=1, space="PSUM") as mps,
            tc.tile_pool(name="tpsum", bufs=2, space="PSUM") as tps,
        ):
            # ---------------- constants / weights in SBUF ----------------
            # DMA issue order matters: the sync queue is FIFO, so conv's
            # stationaries (ahi/alo) and the first event chunks go first.
            ahi = cst.tile([128, 8 * 96], f16)
            alo = cst.tile([128, 8 * 96], f16)
            nc.sync.dma_start(ahi[:].rearrange("p (k m) -> p k m", k=8),
                              ahi_d[:].rearrange("k p m -> p k m"))
            nc.sync.dma_start(alo[:].rearrange("p (k m) -> p k m", k=8),
                              alo_d[:].rearrange("k p m -> p k m"))
            negTH = cst.tile([128, 1], f32)
            nc.vector.memset(negTH[:], -TH)

            # ---------------- conv + temporal filters --------------------
            # a1 free layout: (chunk=(dj,jh,j) 96, b 2, t 60)
            a1 = accp.tile([96, NCH * BT], f32)
            a1v = a1[:].rearrange("p (k b t) -> p k b t", k=NCH, b=BL)
            BLKF = CHB * BL * T
            with tc.tile_pool(name="ev", bufs=1) as evp:
                # events -> SBUF (h on partitions)
                EV = evp.tile([128, NFR * FW], f16)
                ev3 = EV[:].rearrange("p (f w) -> p f w", f=NFR)
                ev6 = EV[:].rearrange("p (b t c r j) -> p b t c r j",
                                      b=BL, t=T, c=2, r=4)
                src = ev_d[:].rearrange("p b t c r j -> p (b t c) (r j)")
                NDMA = 8
                FPD = NFR // NDMA
                for k in range(NDMA):
                    nc.sync.dma_start(
                        ev3[:, k * FPD : (k + 1) * FPD, :],
                        src[:, k * FPD : (k + 1) * FPD, :],
                    )
                csh = cst.tile([128, 4], f32)
                nc.sync.dma_start(csh[:], csh_d[:].rearrange("k p -> p k"))
                wohi = cst.tile([128, 4 * OUT], f16)
                wolo = cst.tile([128, 4 * OUT], f16)
                nc.sync.dma_start(wohi[:].rearrange("p (k m) -> p k m", k=4),
                                  wohi_d[:].rearrange("k p m -> p k m"))
                nc.sync.dma_start(wolo[:].rearrange("p (k m) -> p k m", k=4),
                                  wolo_d[:].rearrange("k p m -> p k m"))
                cso = cst.tile([OUT, 1], f32)
                nc.sync.dma_start(cso[:], cso_d[:].rearrange("(p o) -> p o", o=1))
                ident = cst.tile([128, 128], f32)
                nc.sync.dma_start(ident[:], id_d[:])
                p1 = patp.tile([96, CHB * BL * T], f32)
                p2 = patp.tile([96, CHB * BL * T], f32)
                pds_h = patp.tile([128, 4 * BT], f32)
                pds_o = patp.tile([OUT, BT], f32)
                nc.sync.dma_start(p1[:], p1_d[:])
                nc.sync.dma_start(p2[:], p2_d[:])
                nc.sync.dma_start(pds_h[:], pdh_d[:])
                nc.sync.dma_start(pds_o[:], pdo_d[:])
                for dj in range(3):
                    for b in range(BL):
                        for g in range(NFG):
                            ps = cps.tile([96, FGRP * 32], f32)
                            t0 = g * FGRP
                            first = True
                            for c in range(2):
                                for q in range(4):
                                    r0 = (q + dj - 1) % 4
                                    j0 = (q + dj - 1) // 4
                                    rhs = ev6[:, b, t0 : t0 + FGRP, c, r0,
                                              j0 + 1 : j0 + 33]
                                    for wt in (ahi, alo):
                                        nc.tensor.matmul(
                                            ps[:],
                                            wt[:, (c * 4 + q) * 96 : (c * 4 + q + 1) * 96],
                                            rhs,
                                            start=first,
                                            stop=(c == 1 and q == 3 and wt is alo),
                                        )
                                        first = False
                            dst = a1v[:, dj * 32 : dj * 32 + 32, b, t0 : t0 + FGRP]
                            srcp = ps[:].rearrange("p (t j) -> p j t", t=FGRP)
                            nc.scalar.activation(dst, srcp, F_.Copy,
                                                 scale=1.0 / CONV_SC)
            # EV pool released: the weight-stream pool below reuses its SBUF.
            wp = tc.alloc_tile_pool(name="w", bufs=WPREF)
            wt_tiles = []
            for kk in range(NCH // 2):
                wt = wp.tile([96, 4 * HID], f16, tag="wh")
                nc.sync.dma_start(wt[:], whl_d[kk])
                wt_tiles.append(wt)
            for fb in range(FB):
                blk = slice(fb * BLKF, (fb + 1) * BLKF)
                if debug_taps:
                    nc.sync.dma_start(taps["ec"][:, blk], a1[:, blk])
                a2nb = accp.tile([96, BLKF], f32, tag="a2nb", bufs=2, name="a2nb")
                nc.vector.tensor_tensor_scan(          # a2nb = -a2
                    a2nb[:], p2[:], a1[:, blk], 0.0, op0=A_.mult, op1=A_.subtract)
                nc.vector.tensor_tensor_scan(          # a1 (in place over EC)
                    a1[:, blk], p1[:], a1[:, blk], 0.0, op0=A_.mult, op1=A_.add)
                if debug_taps:
                    nc.sync.dma_start(taps["a2n"][:, blk], a2nb[:])
                nc.vector.tensor_tensor(               # enc = a1 + a2n (in place)
                    a1[:, blk], a1[:, blk], a2nb[:], op=A_.add)
                if debug_taps:
                    nc.sync.dma_start(taps["a1"][:, blk], a1[:, blk])

            # ---------------- m_in LIF scan -> sign spikes ----------------
            Ssg = accp.tile([96, NCH * BT], f16)
            m_in = [stp.tile([96, NCH * BL], f32, tag=f"m_in{i}", name=f"m_in{i}") for i in range(3)]
            x_in = [stp.tile([96, NCH * BL], f32, tag=f"x_in{i}", name=f"x_in{i}") for i in range(2)]
            nc.vector.memset(m_in[0][:], 0.0)
            sgv = Ssg[:].rearrange("p (k b t) -> p k b t", k=NCH, b=BL)
            mv = [m[:].rearrange("p (k b) -> p k b", k=NCH) for m in m_in]
            xv = [x[:].rearrange("p (k b) -> p k b", k=NCH) for x in x_in]
            for t in range(T):
                cur, nxt = t % 3, (t + 1) % 3
                if USE_LIF:
                    nc.vector._custom_dve(
                        LIF_OP, out=mv[nxt], in0=mv[cur],
                        in1=a1v[:, :, :, t], s0=TH, s1=dm)
                else:
                    nc.vector.scalar_tensor_tensor(      # x = (m<TH)*m
                        x_in[t % 2][:], m_in[cur][:], TH, m_in[cur][:],
                        op0=A_.is_lt, op1=A_.mult)
                    nc.vector.scalar_tensor_tensor(      # m' = x*dm + enc[t]
                        mv[nxt], xv[t % 2], dm, a1v[:, :, :, t],
                        op0=A_.mult, op1=A_.add)
                nc.scalar.sign(sgv[:, :, :, t], mv[nxt], bias=negTH[0:96])

            if debug_taps:
                nc.sync.dma_start(taps["sg"][:], Ssg[:])

            # ---------------- hidden projection matmul -------------------
            phi = mps.tile([BT, HID], f32, tag="phi")
            plo = mps.tile([BT, HID], f32, tag="plo")
            for kk in range(NCH // 2):
                wt = wt_tiles[kk]
                for s in range(2):
                    k = 2 * kk + s
                    lhs = Ssg[:, k * BT : (k + 1) * BT]
                    nc.tensor.matmul(phi[:], lhs, wt[:, (2 * s) * HID : (2 * s + 1) * HID],
                                     start=(k == 0), stop=(k == NCH - 1))
                    nc.tensor.matmul(plo[:], lhs, wt[:, (2 * s + 1) * HID : (2 * s + 2) * HID],
                                     start=(k == 0), stop=(k == NCH - 1))
            wp.release()
            projsb = stp.tile([BT, HID], f32)
            nc.scalar.copy(projsb[:], phi[:])
            nc.vector.scalar_tensor_tensor(          # proj += plo * 2^-11
                projsb[:], plo[:], 1.0 / LO_SC, projsb[:], op0=A_.mult, op1=A_.add)
            if debug_taps:
                nc.sync.dma_start(taps["pj"][:], projsb[:])

            # ---------------- transpose proj; c_h scan --------------------
            projT = stp.tile([128, 4 * BT], f32)
            id120 = ident[0:BT, 0:BT]
            for k in range(4):
                pst = tps.tile([128, BT], f32, tag="tp")
                nc.tensor.transpose(pst[:], projsb[:, k * 128 : (k + 1) * 128], id120)
                nc.scalar.activation(                 # evac + colsum bias
                    projT[:, k * BT : (k + 1) * BT], pst[:],
                    F_.Identity, bias=csh[:, k : k + 1], scale=1.0)
            ch = stp.tile([128, 4 * BT], f32)
            nc.vector.tensor_tensor_scan(
                ch[:], pds_h[:], projT[:], 0.0, op0=A_.mult, op1=A_.add)
            if debug_taps:
                nc.sync.dma_start(taps["ch"][:], ch[:])

            # ---------------- m_h LIF scan -------------------------------
            NR = 8
            Ssh = stp.tile([128, 4 * BT], f16)
            m_h = [stp.tile([128, 4 * BL], f32, tag=f"m_h{i}", name=f"m_h{i}") for i in range(NR)]
            x_h = stp.tile([128, 4 * BL], f32)
            nc.vector.memset(m_h[0][:], 0.0)
            chv = ch[:].rearrange("p (k b t) -> p k b t", k=4, b=BL)
            shv = Ssh[:].rearrange("p (k b t) -> p k b t", k=4, b=BL)
            mhv = [m[:].rearrange("p (k b) -> p k b", k=4) for m in m_h]
            for t in range(T):
                cur, nxt = t % NR, (t + 1) % NR
                if USE_LIF:
                    nc.vector._custom_dve(
                        LIF_OP, out=mhv[nxt], in0=mhv[cur],
                        in1=chv[:, :, :, t], s0=TH, s1=dm)
                else:
                    nc.vector.scalar_tensor_tensor(
                        x_h[:], m_h[cur][:], TH, m_h[cur][:], op0=A_.is_lt, op1=A_.mult)
                    nc.vector.scalar_tensor_tensor(
                        mhv[nxt], x_h[:].rearrange("p (k b) -> p k b", k=4),
                        dm, chv[:, :, :, t], op0=A_.mult, op1=A_.add)
                nc.scalar.sign(shv[:, :, :, t], mhv[nxt], bias=negTH[:])
            if debug_taps:
                shf = stp.tile([128, 4 * BT], f32)
                nc.vector.tensor_copy(shf[:], Ssh[:])
                nc.sync.dma_start(taps["sh"][:], shf[:])

            # ---------------- output projection --------------------------
            pho = tps.tile([BT, OUT], f32, tag="po")
            plo2 = tps.tile([BT, OUT], f32, tag="po")
            for k in range(4):
                lhs = Ssh[:, k * BT : (k + 1) * BT]
                nc.tensor.matmul(pho[:], lhs, wohi[:, k * OUT : (k + 1) * OUT],
                                 start=(k == 0), stop=(k == 3))
                nc.tensor.matmul(plo2[:], lhs, wolo[:, k * OUT : (k + 1) * OUT],
                                 start=(k == 0), stop=(k == 3))
            posb = stp.tile([BT, OUT], f32)
            nc.scalar.copy(posb[:], pho[:])
            nc.vector.scalar_tensor_tensor(
                posb[:], plo2[:], 1.0 / LO_SC, posb[:], op0=A_.mult, op1=A_.add)
            pot = tps.tile([OUT, BT], f32, tag="tp")
            nc.tensor.transpose(pot[:], posb[:], id120)
            poT = stp.tile([OUT, BT], f32)
            nc.scalar.activation(poT[:], pot[:], F_.Identity, bias=cso[:], scale=1.0)
            if debug_taps:
                nc.sync.dma_start(taps["po"][:], poT[:])

            co = stp.tile([OUT, BT], f32)
            nc.vector.tensor_tensor_scan(
                co[:], pds_o[:], poT[:], 0.0, op0=A_.mult, op1=A_.add)

            # ---------------- m_o LIF scan + spike count ------------------
            Sso = stp.tile([OUT, BT], f32)
            m_o = [stp.tile([OUT, BL], f32, tag=f"m_o{i}", name=f"m_o{i}") for i in range(NR)]
            x_o = stp.tile([OUT, BL], f32)
            nc.vector.memset(m_o[0][:], 0.0)
            cov = co[:].rearrange("p (b t) -> p b t", b=BL)
            sov = Sso[:].rearrange("p (b t) -> p b t", b=BL)
            for t in range(T):
                cur, nxt = t % NR, (t + 1) % NR
                if USE_LIF:
                    nc.vector._custom_dve(
                        LIF_OP, out=m_o[nxt][:], in0=m_o[cur][:],
                        in1=cov[:, :, t], s0=TH, s1=dm)
                else:
                    nc.vector.scalar_tensor_tensor(
                        x_o[:], m_o[cur][:], TH, m_o[cur][:], op0=A_.is_lt, op1=A_.mult)
                    nc.vector.scalar_tensor_tensor(
                        m_o[nxt][:], x_o[:], dm, cov[:, :, t], op0=A_.mult, op1=A_.add)
                nc.scalar.sign(sov[:, :, t], m_o[nxt][:], bias=negTH[0:OUT])
            # out = (sum_t sgn + T) / (2T)
            accT = stp.tile([OUT, BL], f32)
            nc.vector.tensor_reduce(
                accT[:], Sso[:].rearrange("p (b t) -> p b t", b=BL),
                axis=mybir.AxisListType.X, op=A_.add)
            outsb = stp.tile([OUT, BL], f32)
            nc.vector.tensor_scalar(
                outsb[:], accT[:], 1.0 / (2.0 * T), 0.5, op0=A_.mult, op1=A_.add)
            nc.sync.dma_start(out_d[:].rearrange("b o -> o b"), outsb[:])

    nc.finalize()
    return nc


def _host_prep(events, w_conv, w_hid, w_out):
    ev = np.clip(events, 0, 1).astype(np.float16)          # [B,T,2,H,W] {0,1}
    # residue-shuffled, h-major, margin-padded: evr[h,b,t,c,r,1+j] = ev[b,t,c,h,4j+r]
    evr = np.zeros((128, B, T, 2, 4, 34), np.float16)
    evr[:, :, :, :, :, 1:33] = (
        ev.reshape(B, T, 2, H, 32, 4).transpose(3, 0, 1, 2, 5, 4))

    wc = (np.asarray(w_conv, np.float64) * SCALE * CONV_SC).astype(np.float32)
    A = np.zeros((8, 128, 96), np.float32)
    for c in range(2):
        for q in range(4):
            for di in range(3):
                for i in range(32):
                    for p in range(4):
                        h = 4 * i + di - 1 + p
                        if 0 <= h < 128:
                            A[c * 4 + q, h, di * 32 + i] = wc[c, p, q]
    ahi = A.astype(np.float16)
    alo = (A - ahi.astype(np.float32)).astype(np.float16)

    Wh = (np.asarray(w_hid, np.float64) / 2.0).astype(np.float32)   # [512, 9216]
    WT = np.ascontiguousarray(Wh.T)                                  # [9216, 512]
    djs, js = np.divmod(np.arange(NCH), 32)
    dis, is_ = np.divmod(np.arange(96), 32)
    fmap = ((dis[None, :] * 3 + djs[:, None]) * 1024
            + is_[None, :] * 32 + js[:, None])                       # [NCH, 96]
    whP = WT[fmap]                                                   # [NCH, 96, 512]
    whi = whP.astype(np.float16)
    wlo = ((whP - whi.astype(np.float32)) * LO_SC).astype(np.float16)
    # interleave (hi, lo) per chunk-pair: [NCH//2, 96, 4*HID]
    whl = np.empty((NCH // 2, 96, 4 * HID), np.float16)
    for s in range(2):
        whl[:, :, 2 * s * HID : (2 * s + 1) * HID] = whi[s::2].transpose(0, 1, 2)[
            np.arange(NCH // 2)] if False else whi[2 * np.arange(NCH // 2) + s]
        whl[:, :, (2 * s + 1) * HID : (2 * s + 2) * HID] = wlo[2 * np.arange(NCH // 2) + s]
    csh = Wh.sum(axis=1).astype(np.float32).reshape(4, 128)

    WoC = (np.asarray(w_out, np.float64).T / 2.0).astype(np.float32).reshape(4, 128, OUT)
    wohi = WoC.astype(np.float16)
    wolo = ((WoC - wohi.astype(np.float32)) * LO_SC).astype(np.float16)
    cso = (np.asarray(w_out, np.float64).sum(axis=1) / 2.0).astype(np.float32)

    def seg_pattern(nrow, nseg, decay):
        row = np.full(T, decay, np.float32)
        row[0] = 0.0
        return np.broadcast_to(np.tile(row, nseg), (nrow, nseg * T)).copy()

    shared = dict(ahi=ahi, alo=alo, whl=whl, csh=csh, wohi=wohi, wolo=wolo,
                  cso=cso, ident=np.eye(128, dtype=np.float32),
                  p1=seg_pattern(96, CHB * BL, d1),
                  p2=seg_pattern(96, CHB * BL, d2),
                  pdh=seg_pattern(128, 4 * BL, ds),
                  pdo=seg_pattern(OUT, BL, ds))
    return evr, shared


def _run(events, w_conv, w_hid, w_out, debug_taps=False, trace=False, ncores=NCORES):
    key = ("nc", debug_taps)
    if key not in _CACHE:
        _CACHE[key] = _build_program(debug_taps=debug_taps)
    nc = _CACHE[key]
    ev, shared = _host_prep(events, w_conv, w_hid, w_out)
    in_maps = []
    for c in range(ncores):
        m = {"ev": np.ascontiguousarray(ev[:, c * BL : (c + 1) * BL])}
        m.update(shared)
        in_maps.append(m)
    res = run_bass_kernel_spmd(nc, in_maps, core_ids=list(range(ncores)), trace=trace)
    out = np.concatenate([res.results[c]["out"] for c in range(ncores)], axis=0)
    return out.astype(np.float32), res


def kernel(events, w_conv, w_hid, w_out, batch_size):
    out, _ = _run(np.asarray(events), np.asarray(w_conv),
                  np.asarray(w_hid), np.asarray(w_out))
    return out

